# revision 16
# baseline (speedup 1.0000x reference)
"""Trainium2 Bass kernel for nn_DynamicFeatureGroupingLayer.

Reference computation (B=4096, G=10 groups of S=100 features, M=4 masks,
H=512 hidden):
    mask = entmax(1.1, W_masks)                       # [G,M,S]
    h_t[b,g,m,:] = (x_g[b] * mask[g,m]) @ W_t[g].T    # t in {1,2}
    n_t = layernorm(h_t)
    out[b,g] = sum_m sigmoid(n_1) * relu(n_2)         # [B, G*H]

Strategy (v4):
  * Data-parallel over batch across 8 cores (512 rows each).
  * Host folds the mask into the weights AND mean-centers them over h:
      W~_t[g,m] = mask[g,m,:] * W_t[g];  Wc_t = W~_t - mean_h(W~_t)
    so h_t = x_g @ Wc_t.T is already mean-centered per sample (mean_h is
    linear in x).  No mu columns, no mu copies, no nb smalls on device.
  * LN second moments via Cholesky-Gram in x-space: ss = ||L^T x||^2 with
    L = chol(Wc^T Wc)/sqrt(H)  [S,S], so the reduce yields var directly.
    Square on ACT, chunk-reduce on DVE.
  * Epilogue per (group, mask):
      s   = sigmoid(h1c * rs1)          1 ACT op / mask (scale only)
      w'  = (h2c * rs2) * s             1 STT op / mask (PSUM x SBUF; the
                                        free scalar slot absorbs rs2, and
                                        relu(w') = relu(n2) * s since s > 0)
      v   = relu(w')                    1 batched scalar-free TS (4x mode)
      out = sum_m v                     2 GPSIMD flat contiguous adds
  * smalls per chunk: sd = sqrt(var+eps) [ACT], rs = 1/sd [DVE recip];
    uneven CHUNKS front-load the pipeline; B-units run a 1-deep
    front(matmul+sigmoid)/back(gate+adds) software pipeline so the strict
    per-engine FIFOs never stall on same-unit dependencies.
"""

import numpy as np

B = 4096
INPUT_SIZE = 1000
H = 512
M = 4
S = 100
G = 10
N_CORES = 8
BC = B // N_CORES            # batch rows per core (512)
NBC = BC // 128              # 128-row chunks per core (4)
GRP = 5                      # groups g per weight-load block
CHUNKS = [4, 8, 14, 14]      # units per stats chunk (small first: fast fill)
EPS_LN = 1e-5

MM_DTYPE = "bf16"

_STATE = {}


# --------------------------------------------------------------------------
# host-side preprocessing
# --------------------------------------------------------------------------

def _entmax(alpha, v):
    v = v - np.max(v, axis=-1, keepdims=True)
    e = np.exp(v)
    s = (np.sum(e ** alpha, axis=-1, keepdims=True) + 1e-5) ** (1.0 / alpha)
    return e / s


def _host_prep(x, W_masks, W1, W2):
    """Returns (xt_per_core, W_rhs, L_rhs) as float32 arrays."""
    x = np.asarray(x, np.float32)
    mask = _entmax(1.1, np.asarray(W_masks, np.float64)).astype(np.float64)
    W1 = np.asarray(W1, np.float64)
    W2 = np.asarray(W2, np.float64)

    # W~_t[g,m,h,s] = mask[g,m,s] * W_t[g,h,s], then center over h
    Wt1 = mask[:, :, None, :] * W1[:, None, :, :]        # [G,M,H,S]
    Wt2 = mask[:, :, None, :] * W2[:, None, :, :]
    Wt1 = Wt1 - Wt1.mean(axis=2, keepdims=True)
    Wt2 = Wt2 - Wt2.mean(axis=2, keepdims=True)

    # main rhs: [G, S, M*2*H], col = m*1024 + t*512 + h
    W_rhs = np.stack([Wt1, Wt2], axis=2)                  # [G,M,2,H,S]
    W_rhs = W_rhs.transpose(0, 4, 1, 2, 3).reshape(G, S, M * 2 * H)

    # cholesky of gram matrices, scaled by 1/sqrt(H) so that
    # ||L^T x||^2 = var directly; chunk k = 2m+t at cols k*100:(k+1)*100
    L_rhs = np.zeros((G, S, 8 * S), np.float64)
    for g in range(G):
        for m in range(M):
            for t, Wt in enumerate((Wt1, Wt2)):
                Wm = Wt[g, m]                              # [H,S]
                Gm = Wm.T @ Wm                             # [S,S]
                jit = 1e-9 * np.trace(Gm) / S
                Lm = np.linalg.cholesky(Gm + jit * np.eye(S)) / np.sqrt(H)
                k = 2 * m + t
                L_rhs[g, :, k * S:(k + 1) * S] = Lm

    # x transposed per core: xt[s, g*512 + b] = x[c*512+b, g*100+s]
    xt_cores = []
    for c in range(N_CORES):
        xc = x[c * BC:(c + 1) * BC]                        # [512, 1000]
        xt = np.ascontiguousarray(
            xc.reshape(BC, G, S).transpose(2, 1, 0).reshape(S, G * BC))
        xt_cores.append(xt)

    return xt_cores, W_rhs.astype(np.float32), L_rhs.astype(np.float32)


# --------------------------------------------------------------------------
# tile patch (this walrus build accepts at most ONE sync wait per inst)
# --------------------------------------------------------------------------

def _install_tile_patch():
    import concourse.mybir as mybir
    from concourse.tile import TileContext, ScopedClock

    if getattr(TileContext, "_drain_patched", False):
        return

    def _patched(self, tick_clock, wait_clock):
        nc = self.nc
        probe = nc.sync.nop(hint="drain_waits", nofuse=True)
        wait_clock.add_sem_waits(
            probe.ins, ScopedClock({None: tick_clock.global_clock}))
        si = probe.ins.sync_info
        if si is not None and len(si.on_wait) > 1:
            waits = list(si.on_wait)
            si.on_wait = [waits[0]]
            probe.ins.sync_info = si
            for w in waits[1:]:
                extra = nc.sync.nop(hint="drain_waits_x", nofuse=True)
                extra.ins.sync_info = mybir.SyncInfo(on_wait=[w], on_update=[])
        nc.sync.drain()
        nc.all_engine_barrier()
        popped = nc._tile_sem_poison_stack.pop()
        assert popped is self._sem_poison
        nc.clear_and_free_semaphores(list(self.sems.allocated().values()))
        nc.all_engine_barrier()

    TileContext._drain_and_barrier = _patched

    orig_commit = TileContext._commit_instruction

    def _commit_split(self, inst, lazy_reg_writes=True):
        si = inst.sync_info
        if (
            si is not None
            and len(si.on_wait) > 1
            and inst.engine != mybir.EngineType.Unassigned
        ):
            waits = list(si.on_wait)
            for w in waits[:-1]:
                nop = mybir.InstNoOp(
                    name=self.nc.get_next_instruction_name(),
                    engine=inst.engine,
                    ins=[],
                    outs=[],
                    sync_info=mybir.SyncInfo(on_wait=[w], on_update=[]),
                )
                orig_commit(self, nop, lazy_reg_writes=False)
            si.on_wait = [waits[-1]]
            inst.sync_info = si
        return orig_commit(self, inst, lazy_reg_writes)

    TileContext._commit_instruction = _commit_split
    TileContext._drain_patched = True


# --------------------------------------------------------------------------
# device kernel
# --------------------------------------------------------------------------

def _build_program():
    import concourse.bass as bass
    import concourse.mybir as mybir
    import concourse.tile as tile

    _install_tile_patch()
    dt = mybir.dt
    AF = mybir.ActivationFunctionType
    OP = mybir.AluOpType
    AX = mybir.AxisListType
    mm_dt = {"f32r": dt.float32r, "f32": dt.float32, "bf16": dt.bfloat16}[MM_DTYPE]
    f16 = dt.bfloat16

    nc = bass.Bass()
    xt0_d = nc.declare_dram_parameter("xt0", [S, BC], mm_dt, isOutput=False)
    xt_d = nc.declare_dram_parameter("xt", [S, (G - 1) * BC], mm_dt, isOutput=False)
    w_d = nc.declare_dram_parameter("w", [G, S, M * 2 * H], mm_dt, isOutput=False)
    l_d = nc.declare_dram_parameter("l", [G, S, 8 * S], mm_dt, isOutput=False)
    y_d = nc.declare_dram_parameter("y", [BC, G * H], f16, isOutput=True)

    units_all = [(g, bc) for g in range(G) for bc in range(NBC)]
    n_chunks = len(CHUNKS)
    chunk_base = [sum(CHUNKS[:i]) for i in range(n_chunks)]
    assert sum(CHUNKS) == len(units_all)

    with tile.TileContext(nc) as tc:
        with (
            tc.tile_pool(name="xpool", bufs=1) as xpool,
            tc.tile_pool(name="wpool", bufs=10) as wpool,
            tc.tile_pool(name="lpool", bufs=10) as lpool,
            tc.tile_pool(name="hpsum", bufs=6, space="PSUM") as hpsum,
            tc.tile_pool(name="zpsum", bufs=1, space="PSUM") as zpsum,
            tc.tile_pool(name="spool", bufs=6) as spool,
            tc.tile_pool(name="gpool", bufs=6) as gpool,
            tc.tile_pool(name="vpool", bufs=6) as vpool,
            tc.tile_pool(name="prpool", bufs=4) as prpool,
            tc.tile_pool(name="accpool", bufs=4) as accpool,
            tc.tile_pool(name="ppool", bufs=4) as ppool,
            tc.tile_pool(name="statpool", bufs=2) as statpool,
        ):
            eps_sb = xpool.tile([128, 1], dt.float32, tag="eps")
            nc.vector.memset(eps_sb[:], EPS_LN)
            xt0_sb = xpool.tile([S, BC], mm_dt, tag="xt0")
            xt_sb = xpool.tile([S, (G - 1) * BC], mm_dt, tag="xt")

            def xch(g, bc):
                if g == 0:
                    return xt0_sb[:, bc * 128:(bc + 1) * 128]
                gg = g - 1
                return xt_sb[:, gg * BC + bc * 128: gg * BC + (bc + 1) * 128]

            wl_sbs = {}

            def load_weights(blk):
                # L tensors are small and gate the stats prologue: load the
                # block's L tiles first, then (block 0 only) xt, then the W.
                gs = [blk * GRP + i for i in range(GRP)]
                if blk == 0:
                    l = lpool.tile([S, 8 * S], mm_dt, tag="l", name="lsb0")
                    nc.sync.dma_start(l[:], l_d[0])
                    wl_sbs[0] = [None, l]
                    nc.sync.dma_start(xt0_sb[:], xt0_d[:])
                    w0 = wpool.tile([S, M * 2 * H], mm_dt, tag="w", name="wsb0")
                    nc.sync.dma_start(w0[:], w_d[0])
                    wl_sbs[0][0] = w0
                for g in gs:
                    if g == 0:
                        continue
                    l = lpool.tile([S, 8 * S], mm_dt, tag="l", name=f"lsb{g}")
                    nc.sync.dma_start(l[:], l_d[g])
                    wl_sbs[g] = [None, l]
                if blk == 0:
                    nc.sync.dma_start(xt_sb[:], xt_d[:])
                for g in gs:
                    if g == 0:
                        continue
                    w = wpool.tile([S, M * 2 * H], mm_dt, tag="w", name=f"wsb{g}")
                    nc.sync.dma_start(w[:], w_d[g])
                    wl_sbs[g][0] = w

            stats = {}

            def alloc_stats(c):
                ss = statpool.tile([128, 8 * CHUNKS[c]], dt.float32, tag="ss")
                stats[c] = {"ss": ss}

            def emit_A_unit(c, u):
                g, bc = units_all[chunk_base[c] + u]
                st = stats[c]
                l_sb = wl_sbs[g][1]
                psq = ppool.tile([128, 800], f16, tag="p")
                # prologue chunk: alternate za into the (still idle) h ring so
                # the first stats units pipeline 2-deep instead of serializing
                if c == 0 and u % 2 == 1:
                    za_a = hpsum.tile([128, 512], dt.float32, tag="h",
                                      name="za_a")
                    za_b = hpsum.tile([128, 512], dt.float32, tag="h",
                                      name="za_b")
                    nc.tensor.matmul(za_a[:, 0:400], xch(g, bc), l_sb[:, 0:400])
                    nc.tensor.matmul(za_b[:, 0:400], xch(g, bc), l_sb[:, 400:800])
                    nc.scalar.activation(psq[:, 0:400], za_a[:, 0:400], AF.Square)
                    nc.scalar.activation(psq[:, 400:800], za_b[:, 0:400], AF.Square)
                else:
                    za = zpsum.tile([128, 1024], dt.float32, tag="za")
                    nc.tensor.matmul(za[:, 0:400], xch(g, bc), l_sb[:, 0:400])
                    nc.tensor.matmul(za[:, 512:912], xch(g, bc), l_sb[:, 400:800])
                    # one Square over both halves via a strided 3D view
                    zview = za[:].rearrange("p (h r) -> p h r", h=2)[:, :, 0:400]
                    nc.scalar.activation(
                        psq[:].rearrange("p (h r) -> p h r", h=2), zview,
                        AF.Square)
                nc.vector.reduce_sum(
                    st["ss"][:, u * 8:(u + 1) * 8],
                    psq[:].rearrange("p (q r) -> p q r", r=S),
                    axis=AX.X)

            def emit_smalls(c):
                st = stats[c]
                # ss is already var (L scaled by 1/sqrt(H)); sd = sqrt(var+eps)
                sd = statpool.tile([128, 8 * CHUNKS[c]], dt.float32, tag="sd")
                nc.scalar.activation(sd[:], st["ss"][:], AF.Sqrt, bias=eps_sb[:])
                rs = statpool.tile([128, 8 * CHUNKS[c]], dt.float32, tag="rs")
                nc.vector.reciprocal(rs[:], sd[:])
                st["rs"] = rs

            def emit_B_front(c, u):
                """Matmuls + sigmoids for unit u; returns state for the back."""
                g, bc = units_all[chunk_base[c] + u]
                st = stats[c]
                w_sb = wl_sbs[g][0]
                rs = st["rs"]
                s_all = spool.tile([128, M, H], f16, tag="s")
                h2ps = []
                for p in range(2):
                    m0 = 2 * p
                    for i in range(2):
                        m = m0 + i
                        h1m = hpsum.tile([128, 512], dt.float32, tag="h",
                                         name=f"h1m{m}")
                        nc.tensor.matmul(
                            h1m[:], xch(g, bc),
                            w_sb[:, m * 2 * H: m * 2 * H + H])
                        # sigmoid(h1c * rs1), no bias needed (pre-centered)
                        c1 = slice(u * 8 + 2 * m, u * 8 + 2 * m + 1)
                        nc.scalar.activation(
                            s_all[:, m], h1m[:], AF.Sigmoid, scale=rs[:, c1])
                    for i in range(2):
                        m = m0 + i
                        h2m = hpsum.tile([128, 512], dt.float32, tag="h",
                                         name=f"h2m{m}")
                        nc.tensor.matmul(
                            h2m[:], xch(g, bc),
                            w_sb[:, m * 2 * H + H: m * 2 * H + 2 * H])
                        h2ps.append(h2m)
                return (c, u, g, bc, s_all, h2ps)

            def emit_B_back(state):
                c, u, g, bc, s_all, h2ps = state
                st = stats[c]
                rs = st["rs"]
                w_all = gpool.tile([128, M, H], f16, tag="g")
                v_all = vpool.tile([128, M, H], f16, tag="v")
                # w' = (h2c * rs2) * s per mask, one STT op (PSUM x SBUF);
                # relu(w') = relu(n2) * s since s > 0
                for m in range(M):
                    c2 = slice(u * 8 + 2 * m + 1, u * 8 + 2 * m + 2)
                    nc.vector.scalar_tensor_tensor(
                        w_all[:, m], h2ps[m][:],
                        rs[:, c2], s_all[:, m],
                        op0=OP.mult, op1=OP.mult)
                # v = relu(w') for all four masks in one scalar-free TS op
                nc.vector.tensor_scalar(
                    v_all[:].rearrange("p m h -> p (m h)"),
                    w_all[:].rearrange("p m h -> p (m h)"),
                    0.0, None, op0=OP.max)
                # mask-sum: two flat contiguous adds on gpsimd
                pr = prpool.tile([128, 2 * H], f16, tag="pr")
                nc.gpsimd.tensor_tensor(
                    pr[:],
                    v_all[:, 0:2].rearrange("p m h -> p (m h)"),
                    v_all[:, 2:4].rearrange("p m h -> p (m h)"),
                    op=OP.add)
                acc = accpool.tile([128, H], f16, tag="acc")
                nc.gpsimd.tensor_tensor(
                    acc[:], pr[:, 0:H], pr[:, H:2 * H], op=OP.add)
                nc.sync.dma_start(
                    y_d[bc * 128:(bc + 1) * 128, g * H:(g + 1) * H], acc[:])

            # ---- software pipeline: stats chunk c+1 interleaved into the
            # epilogue of chunk c ----
            load_weights(0)
            load_weights(1)
            alloc_stats(0)
            for u in range(CHUNKS[0]):
                emit_A_unit(0, u)
            emit_smalls(0)
            pending = None
            for c in range(n_chunks):
                nb = CHUNKS[c]
                if c + 1 < n_chunks:
                    alloc_stats(c + 1)
                    na = CHUNKS[c + 1]
                    denom = max(nb - 3, 1)
                else:
                    na = 0
                    denom = 1
                ai = 0
                for u in range(nb):
                    front = emit_B_front(c, u)
                    if pending is not None:
                        emit_B_back(pending)
                    pending = front
                    # next chunk's stats units, front-loaded so the smalls
                    # land a few B-units before the chunk boundary
                    while ai < na and ai * denom < (u + 1) * na:
                        emit_A_unit(c + 1, ai)
                        ai += 1
                        if ai == na:
                            emit_smalls(c + 1)
            if pending is not None:
                emit_B_back(pending)

    return nc


def _get_state():
    if "nc" not in _STATE:
        _STATE["nc"] = _build_program()
    return _STATE["nc"]


# --------------------------------------------------------------------------
# public entry point
# --------------------------------------------------------------------------

LAST_RESULTS = None


def kernel(x, W_masks, W1, W2, ln1_w, ln1_b, ln2_w, ln2_b):
    global LAST_RESULTS
    import ml_dtypes
    from concourse.bass_utils import run_bass_kernel_spmd

    assert np.allclose(np.asarray(ln1_w), 1.0) and np.allclose(np.asarray(ln2_w), 1.0) \
        and np.allclose(np.asarray(ln1_b), 0.0) and np.allclose(np.asarray(ln2_b), 0.0), \
        "kernel compiled for identity layernorm affine params"

    xt_cores, W_rhs, L_rhs = _host_prep(x, W_masks, W1, W2)
    np_dt = {"f32r": np.float32, "f32": np.float32,
             "bf16": ml_dtypes.bfloat16}[MM_DTYPE]
    W_rhs = W_rhs.astype(np_dt)
    L_rhs = L_rhs.astype(np_dt)

    nc = _get_state()
    in_maps = [
        {"xt0": np.ascontiguousarray(xt_cores[c][:, :BC]).astype(np_dt),
         "xt": np.ascontiguousarray(xt_cores[c][:, BC:]).astype(np_dt),
         "w": W_rhs, "l": L_rhs}
        for c in range(N_CORES)
    ]
    res = run_bass_kernel_spmd(nc, in_maps, list(range(N_CORES)))
    LAST_RESULTS = res
    out = np.concatenate([res.results[c]["y"] for c in range(N_CORES)], axis=0)
    return out.astype(np.float32)
